# revision 10
# baseline (speedup 1.0000x reference)
"""Trainium2 Bass kernel for nn_BatchProgramClassifier.

Reference computation (B=64, L=64, NPT=127, D=128, VOCAB=30000, LABELS=30):
  1. e = emb[tokens] @ Wc + bc                     per tree node
  2. h = bottom-up subtree sums of e (heap tree)   [B, L, NPT, D]
  3. enc = relu(max over nodes of h)               [B, L, D]
  4. masked single-head self-attention over L      [B, L, D]
  5. logits = (max over L) @ Wl + bl               [B, LABELS]

Sharding: data-parallel over batch, 8 batches per core across 8 cores.

The scarce resource is dma_gather descriptor generation on the Q7 cores
(~3.9 ns/row per queue-pair; queue q runs on cores 2q/2q+1, and different
queues' generation overlaps). The kernel keeps 4 gathers in flight on the
4 SWDGE queues and paces everything else to consume at that rate:
  - 16 gathers of 4096 rows (32 trees), node-major columns
    (col = node*32 + tree) so tree-level DVE ops run in 2x perf mode.
  - Per 1024 columns: 2 Wc matmuls into a 2-bank PSUM tile (2 tiles
    rotating), one ACT eviction folding the +bc bias.
  - Subtree sums + running node max fused in one bottom-up DVE pass per
    batch (2 chunks); the root step fuses the ReLU.
  - Attention runs per batch as soon as its enc columns exist; v-rows
    come from a matmul with lhsT=enc so only attn needs a transpose.
"""

import math

import numpy as np

B, L, NPT, D_TREE = 64, 64, 127, 7
VOCAB, D, LABELS = 30000, 128, 30
NCORES = 8
BC = B // NCORES  # batches per core
TREES = BC * L  # trees per core
CHUNK_TREES = 32  # trees per gather chunk
NCHUNKS = TREES // CHUNK_TREES  # 16
NIDX_CHUNK = CHUNK_TREES * 128  # 4096
NIDX_TOTAL = TREES * 128  # 65536

_CACHE = {}


def _build_nc():
    import concourse.bacc as bacc
    import concourse.mybir as mybir
    import concourse.tile as tile
    from concourse.library_config import mlp

    f32 = mybir.dt.float32
    f16 = mybir.dt.float16
    nc = bacc.Bacc(
        "TRN2",
        target_bir_lowering=False,
        debug=False,
        num_devices=NCORES,
        num_swdge_queues=4,
    )

    emb_d = nc.dram_tensor("emb", [VOCAB, D], f16, kind="ExternalInput")
    idx_d = nc.dram_tensor(
        "idxs", [128, NIDX_TOTAL // 16], mybir.dt.int16, kind="ExternalInput"
    )
    mask_d = nc.dram_tensor("mask", [L, BC * L], mybir.dt.int32, kind="ExternalInput")
    wc_d = nc.dram_tensor("wc", [D, D], f32, kind="ExternalInput")
    bcv_d = nc.dram_tensor("bcv", [D, 1], f32, kind="ExternalInput")
    wq_d = nc.dram_tensor("wq", [D, D], f32, kind="ExternalInput")
    wk_d = nc.dram_tensor("wk", [D, D], f32, kind="ExternalInput")
    wv_d = nc.dram_tensor("wv", [D, D], f32, kind="ExternalInput")
    wo_d = nc.dram_tensor("wo", [D, D], f32, kind="ExternalInput")
    wl_d = nc.dram_tensor("wl", [D, LABELS], f32, kind="ExternalInput")
    blb_d = nc.dram_tensor("blb", [BC, LABELS], f32, kind="ExternalInput")
    ident_d = nc.dram_tensor("ident", [64, 64], f32, kind="ExternalInput")
    out_d = nc.dram_tensor("out", [BC, LABELS], f32, kind="ExternalOutput")

    inv_sqrt_d = 1.0 / math.sqrt(float(D))
    mx = mybir.AluOpType.max

    with tile.TileContext(nc) as tc:
        with (
            tc.tile_pool(name="const", bufs=1) as cpool,
            tc.tile_pool(name="epool", bufs=8) as epool,
            tc.tile_pool(name="eblk", bufs=3) as eblkpool,
            tc.tile_pool(name="mpool", bufs=2) as mpool,
            tc.tile_pool(name="tpsum", bufs=3, space="PSUM") as tpsum,
            tc.tile_pool(name="apool", bufs=2) as apool,
            tc.tile_pool(name="apsum", bufs=1, space="PSUM") as apsum,
        ):
            nc.gpsimd.load_library(mlp)

            # idx for chunk 0 first so gather 0 can start ASAP
            idx_t = cpool.tile([128, NIDX_TOTAL // 16], mybir.dt.int16, tag="idxs")
            idx_cols = NIDX_CHUNK // 16  # 256
            nc.sync.dma_start(out=idx_t[:, :idx_cols], in_=idx_d[:, :idx_cols])

            def gather(c, et):
                nc.gpsimd.dma_gather(
                    et[:],
                    emb_d[:],
                    idx_t[:, c * idx_cols : (c + 1) * idx_cols],
                    NIDX_CHUNK,
                    NIDX_CHUNK,
                    D,
                    transpose=True,
                    single_packet=False,
                    queue_num=c % 4,
                )

            et0 = epool.tile([128, 1, NIDX_CHUNK], f16, tag="et")
            gather(0, et0)
            nc.sync.dma_start(out=idx_t[:, idx_cols:], in_=idx_d[:, idx_cols:])

            def load_const(dram, shape, dtype):
                t = cpool.tile(shape, dtype, tag=dram.name)
                if dtype == dram.dtype:
                    nc.sync.dma_start(out=t[:], in_=dram[:])
                else:
                    raw = cpool.tile(shape, dram.dtype, tag=dram.name + "_raw")
                    nc.sync.dma_start(out=raw[:], in_=dram[:])
                    nc.scalar.copy(out=t[:], in_=raw[:])
                return t

            wc_t = load_const(wc_d, [D, D], f16)
            bcv_t = load_const(bcv_d, [D, 1], f32)
            wq_t = load_const(wq_d, [D, D], f16)
            wk_t = load_const(wk_d, [D, D], f16)
            wv_t = load_const(wv_d, [D, D], f16)
            wo_t = load_const(wo_d, [D, D], f16)
            wl_t = load_const(wl_d, [D, LABELS], f16)
            blb_t = load_const(blb_d, [BC, LABELS], f32)
            ident_t = load_const(ident_d, [64, 64], f32)
            mask_t = load_const(mask_d, [L, BC * L], mybir.dt.int32)

            # additive mask: 0 where mask>0, -1e9 where mask==0
            maskf = cpool.tile([L, BC * L], f32, tag="maskf")
            nc.vector.tensor_copy(out=maskf[:], in_=mask_t[:])
            nmask = cpool.tile([L, BC * L], f32, tag="nmask")
            nc.vector.tensor_scalar(
                nmask[:], maskf[:], 1e9, -1e9, mybir.AluOpType.mult, mybir.AluOpType.add
            )

            enc = cpool.tile([D, TREES], f16, tag="enc")  # enc^T, col = tree
            pooled = cpool.tile([D, BC], f16, tag="pooled")

            for b in range(BC):
                eb = eblkpool.tile([128, 2 * NIDX_CHUNK], f16, tag="eb")
                for g in range(2):
                    c = 2 * b + g
                    if c == 0:
                        et = et0
                    else:
                        et = epool.tile([128, 1, NIDX_CHUNK], f16, tag="et")
                        gather(c, et)
                    for j in range(NIDX_CHUNK // 1024):
                        pp = tpsum.tile([128, 1024], f32, tag="pp")
                        for k in range(2):
                            nc.tensor.matmul(
                                pp[:, k * 512 : (k + 1) * 512],
                                lhsT=wc_t[:],
                                rhs=et[
                                    :, 0, j * 1024 + k * 512 : j * 1024 + (k + 1) * 512
                                ],
                                start=True,
                                stop=True,
                            )
                        nc.scalar.activation(
                            eb[:, g * NIDX_CHUNK + j * 1024 : g * NIDX_CHUNK + (j + 1) * 1024],
                            pp[:],
                            mybir.ActivationFunctionType.Identity,
                            bias=bcv_t[:],
                            scale=1.0,
                        )

                # ---- fused subtree sums + node max over the batch's 64 trees ----
                ebv = eb.rearrange("p (g n t) -> p g n t", g=2, n=128, t=CHUNK_TREES)
                m = mpool.tile([128, 2, 64, CHUNK_TREES], f16, tag="m")
                enc_v = enc[:, b * L : (b + 1) * L].rearrange(
                    "p (g o t) -> p g o t", g=2, o=1, t=CHUNK_TREES
                )
                for lvl in range(D_TREE - 2, -1, -1):
                    s, cnt = 2**lvl - 1, 2**lvl
                    ch = ebv[:, :, 2 * s + 1 : 2 * s + 1 + 2 * cnt, :].rearrange(
                        "p g (k two) t -> p g k two t", two=2
                    )
                    tmp = mpool.tile([128, 2, 32, CHUNK_TREES], f16, tag="tmp")
                    nc.vector.tensor_add(
                        out=tmp[:, :, :cnt, :],
                        in0=ch[:, :, :, 0, :],
                        in1=ch[:, :, :, 1, :],
                    )
                    mtmp = mpool.tile([128, 2, 32, CHUNK_TREES], f16, tag="mtmp")
                    if lvl == D_TREE - 2:
                        mch = ch  # children are leaves; their h is their max
                    else:
                        mch = m[:, :, 2 * s + 1 : 2 * s + 1 + 2 * cnt, :].rearrange(
                            "p g (k two) t -> p g k two t", two=2
                        )
                    nc.vector.tensor_max(
                        out=mtmp[:, :, :cnt, :],
                        in0=mch[:, :, :, 0, :],
                        in1=mch[:, :, :, 1, :],
                    )
                    nc.vector.tensor_add(
                        out=ebv[:, :, s : s + cnt, :],
                        in0=ebv[:, :, s : s + cnt, :],
                        in1=tmp[:, :, :cnt, :],
                    )
                    if lvl == 0:
                        nc.vector.scalar_tensor_tensor(
                            out=enc_v,
                            in0=ebv[:, :, 0:1, :],
                            scalar=0.0,
                            in1=mtmp[:, :, 0:1, :],
                            op0=mx,
                            op1=mx,
                        )
                    else:
                        nc.vector.tensor_max(
                            out=m[:, :, s : s + cnt, :],
                            in0=ebv[:, :, s : s + cnt, :],
                            in1=mtmp[:, :, :cnt, :],
                        )

                # ---- attention for this batch ----
                enc_b = enc[:, b * L : (b + 1) * L]
                ab0 = apsum.tile([128, 512], f32, tag="ab0")
                ab1 = apsum.tile([128, 512], f32, tag="ab1")
                qp = ab0[:, 0:64]
                kp = ab0[:, 64:128]
                vrp = ab0[:64, 128:256]
                scp = ab1[:64, 0:64]
                atp = ab1[:64, 64:128]
                opp = ab1[:, 128:192]
                o2p = ab1[:, 192:256]

                nc.tensor.matmul(qp, lhsT=wq_t[:], rhs=enc_b, start=True, stop=True)
                qs = apool.tile([D, L], f16, tag="qs")
                nc.scalar.mul(qs[:], qp, inv_sqrt_d)
                nc.tensor.matmul(kp, lhsT=wk_t[:], rhs=enc_b, start=True, stop=True)
                ks = apool.tile([D, L], f16, tag="ks")
                nc.scalar.copy(out=ks[:], in_=kp)
                # v rows directly: (enc_b).T @ Wv = [64 tok, 128 D]
                nc.tensor.matmul(vrp, lhsT=enc_b, rhs=wv_t[:], start=True, stop=True)
                vrs = apool.tile([L, D], f16, tag="vrs")
                nc.scalar.copy(out=vrs[:], in_=vrp)

                nc.tensor.matmul(scp, lhsT=qs[:], rhs=ks[:], start=True, stop=True)
                # scores are bounded (|s| < 1), so no row-max subtraction is
                # needed for a stable softmax; masked entries exp to 0.
                sm = apool.tile([L, L], f32, tag="sm")
                nc.vector.tensor_add(
                    out=sm[:], in0=scp, in1=nmask[:, b * L : (b + 1) * L]
                )
                ex = apool.tile([L, L], f32, tag="ex")
                nc.scalar.activation(ex[:], sm[:], mybir.ActivationFunctionType.Exp)
                rsum = apool.tile([L, 1], f32, tag="rsum")
                nc.vector.reduce_sum(
                    out=rsum[:], in_=ex[:], axis=mybir.AxisListType.X
                )
                rinv = apool.tile([L, 1], f32, tag="rinv")
                nc.vector.reciprocal(rinv[:], rsum[:])
                attn = apool.tile([L, L], f32, tag="attn")
                nc.vector.tensor_mul(
                    out=attn[:], in0=ex[:], in1=rinv[:].to_broadcast((L, L))
                )

                nc.tensor.transpose(atp, attn[:], ident_t[:])
                ats = apool.tile([L, L], f16, tag="ats")
                nc.scalar.copy(out=ats[:], in_=atp)
                nc.tensor.matmul(opp, lhsT=vrs[:], rhs=ats[:], start=True, stop=True)
                oss = apool.tile([D, L], f16, tag="oss")
                nc.scalar.copy(out=oss[:], in_=opp)
                nc.tensor.matmul(o2p, lhsT=wo_t[:], rhs=oss[:], start=True, stop=True)
                nc.vector.reduce_max(
                    out=pooled[:, b : b + 1], in_=o2p, axis=mybir.AxisListType.X
                )

            # ---- logits ----
            lgp = apsum.tile([BC, 64], f32, tag="ab1")
            nc.tensor.matmul(
                lgp[:, :LABELS], lhsT=pooled[:], rhs=wl_t[:], start=True, stop=True
            )
            outs = apool.tile([BC, LABELS], f32, tag="outs")
            nc.vector.tensor_add(out=outs[:], in0=lgp[:, :LABELS], in1=blb_t[:])
            nc.sync.dma_start(out=out_d[:], in_=outs[:])

    nc.compile()
    return nc


def _get_nc():
    if "nc" not in _CACHE:
        _CACHE["nc"] = _build_nc()
    return _CACHE["nc"]


def kernel(tokens, mask, emb, Wc, bc, Wq, Wk, Wv, Wo, Wl, bl, _trace=False, _tmpdir=None):
    from concourse.bass_utils import run_bass_kernel_spmd

    tokens = np.asarray(tokens)
    mask = np.asarray(mask)
    emb16 = np.asarray(emb, dtype=np.float32).astype(np.float16)

    blb = np.tile(np.asarray(bl, np.float32)[None, :], (BC, 1))

    common = {
        "emb": emb16,
        "wc": np.asarray(Wc, np.float32),
        "bcv": np.asarray(bc, np.float32).reshape(D, 1),
        "wq": np.asarray(Wq, np.float32),
        "wk": np.asarray(Wk, np.float32),
        "wv": np.asarray(Wv, np.float32),
        "wo": np.asarray(Wo, np.float32),
        "wl": np.asarray(Wl, np.float32),
        "blb": blb,
        "ident": np.eye(64, dtype=np.float32),
    }

    in_maps = []
    for c in range(NCORES):
        tok_c = np.asarray(tokens[c * BC : (c + 1) * BC]).reshape(TREES, NPT)
        # node-major within each chunk: col = n*CHUNK_TREES + t (n=127 is pad)
        blocks = []
        for ch in range(NCHUNKS):
            blk = tok_c[ch * CHUNK_TREES : (ch + 1) * CHUNK_TREES]
            padded = np.zeros((128, CHUNK_TREES), dtype=tok_c.dtype)
            padded[:NPT, :] = blk.T
            blocks.append(padded.reshape(-1))
        idx_lin = np.concatenate(blocks)
        idx_arr = np.tile(
            idx_lin.astype(np.int16).reshape(-1, 16).T, (8, 1)
        )  # [128, NIDX_TOTAL/16]
        mask_c = (
            np.asarray(mask[c * BC : (c + 1) * BC], np.int32)
            .transpose(1, 0, 2)
            .reshape(L, BC * L)
        )
        in_maps.append({**common, "idxs": idx_arr, "mask": mask_c})

    nc = _get_nc()
    res = run_bass_kernel_spmd(
        nc, in_maps, core_ids=list(range(NCORES)), trace=_trace, tmpdir=_tmpdir
    )
    out = np.concatenate([r["out"] for r in res.results], axis=0)  # [B, LABELS]
    if _trace:
        return out, res
    return out
